# revision 34
# baseline (speedup 1.0000x reference)
"""Trainium2 Bass kernel for DiffGeomPropsApprox (within-batch uv-space 16-NN
-> neighborhood covariance of X -> descending symmetric-3x3 eigenvalues).

Sharding: data-parallel over batch B=8, one batch per NeuronCore (8 cores).

Per-core algorithm (M=4096 points):
  * negdm[i,j] = -((u_i-u_j)^2 + (v_i-v_j)^2), computed with the exact same
    f32 roundings as the reference (sub -> square -> add, negation is exact),
    so the NN ranking is bit-identical to the reference's top_k.
  * per query row: DVE max8 -> match_replace -> max8 gives t16 = the 16th
    largest negdm value; the selection mask is w = (negdm >= t16) in bf16.
  * mask transposed via DMA x-bar transpose (idle DMA engines), cov sums via
    PE: feature-stationary matmuls (18 features = 9 cols x bf16 hi/lo; the
    0/1 mask is exact in bf16) accumulated in PSUM as [18, q].
  * cov = S2 - outer(S1)/16; closed-form symmetric 3x3 eigenvalues
    (trigonometric method; acos/cos built from Arctan/Sin/Sqrt LUTs).
"""

from contextlib import ExitStack

import numpy as np

import concourse.bass as bass
import concourse.tile as tile
from concourse import bacc, mybir
from concourse.alu_op_type import AluOpType
from concourse.bass_utils import run_bass_kernel_spmd

F32 = mybir.dt.float32
BF16 = mybir.dt.bfloat16
I32 = mybir.dt.int32
AF = mybir.ActivationFunctionType
OP = AluOpType

P = 128
K = 16
GROUP = 2           # query tiles per matmul group
NEG_BIG = -3.0e38
PI = float(np.pi)


def _emit(ctx: ExitStack, tc, out_ap, x_ap, uv_ap, M: int, ident18):
    nc = tc.nc
    T = M // P          # number of 128-row tiles
    NF = 18             # 9 features x (hi, lo)
    NDMA = min(8, T)    # split slow strided DMAs this many ways

    const = ctx.enter_context(tc.tile_pool(name="const", bufs=1))
    work = ctx.enter_context(tc.tile_pool(name="work", bufs=2))
    small = ctx.enter_context(tc.tile_pool(name="small", bufs=4))
    psum = ctx.enter_context(tc.tile_pool(name="psum", bufs=2, space="PSUM"))
    epool = ctx.enter_context(tc.tile_pool(name="eig", bufs=1))

    # ---- broadcast candidate coords across partitions (doubling DMAs) ----
    u_b = const.tile([P, M], F32, tag="u_b")
    v_b = const.tile([P, M], F32, tag="v_b")
    uv_t = uv_ap.rearrange("m k -> k m")
    nc.sync.dma_start(u_b[0:1, :], uv_t[0:1, :])
    nc.sync.dma_start(v_b[0:1, :], uv_t[1:2, :])
    k = 1
    while k < P:
        nc.sync.dma_start(u_b[k:2 * k, :], u_b[0:k, :])
        nc.sync.dma_start(v_b[k:2 * k, :], v_b[0:k, :])
        k *= 2

    # slab layouts [P, T, k]: point m = t*128 + p  (strided loads, split)
    uv_slab = const.tile([P, T, 2], F32, tag="uv_slab")
    uv_r = uv_ap.rearrange("(t p) k -> p t k", p=P)
    x_slab = const.tile([P, T, 3], F32, tag="x_slab")
    x_r = x_ap.rearrange("(t p) k -> p t k", p=P)
    tchunk = T // NDMA
    for d in range(NDMA):
        sl = slice(d * tchunk, (d + 1) * tchunk)
        nc.sync.dma_start(uv_slab[:, sl, :], uv_r[:, sl, :])
        nc.sync.dma_start(x_slab[:, sl, :], x_r[:, sl, :])
    nuv = const.tile([P, T, 2], F32, tag="nuv")
    nc.vector.tensor_scalar(out=nuv[:], in0=uv_slab[:], scalar1=-1.0,
                            scalar2=None, op0=OP.mult)

    # ---- features: [x y z x2 y2 z2 xy xz yz] as bf16 hi/lo ----
    pairs = [(0, 0), (1, 1), (2, 2), (0, 1), (0, 2), (1, 2)]
    fsl = work.tile([P, T, 9], F32, tag="fsl", name="fsl", bufs=1)
    nc.vector.tensor_copy(fsl[:, :, 0:3], x_slab[:])
    for i, (a, b) in enumerate(pairs):
        nc.vector.tensor_tensor(out=fsl[:, :, 3 + i], in0=x_slab[:, :, a],
                                in1=x_slab[:, :, b], op=OP.mult)
    fbf = const.tile([P, T, NF], BF16, tag="fbf")
    nc.vector.tensor_copy(fbf[:, :, 0:9], fsl[:])
    fhi32 = work.tile([P, T, 9], F32, tag="fhi32", name="fhi32", bufs=1)
    nc.vector.tensor_copy(fhi32[:], fbf[:, :, 0:9])
    nc.vector.tensor_tensor(out=fbf[:, :, 9:18], in0=fsl[:], in1=fhi32[:],
                            op=OP.subtract)

    cov = const.tile([P, T, NF], F32, tag="cov")

    # per-feature totals sum_c F[c, f] (for the +-1 sign-mask trick):
    # acc = sum_sel f - sum_unsel f  =>  sum_sel = (acc + ftot) / 2
    SPLIT = (11 * T) // 16          # chunks masked +-1 on ACT; rest {2,0} DVE
    ones_c = const.tile([P, 1], BF16, tag="ones_c")
    nc.gpsimd.memset(ones_c[:], 1.0)
    ftot_ps = psum.tile([NF, 1], F32, tag="ftot", name="ftot_ps", bufs=1)
    for j in range(SPLIT):
        nc.tensor.matmul(ftot_ps[:], lhsT=fbf[:, j, :], rhs=ones_c[:],
                         start=(j == 0), stop=(j == SPLIT - 1))
    ftot_h = const.tile([NF, 1], F32, tag="ftot_h")
    nc.vector.tensor_scalar(out=ftot_h[:], in0=ftot_ps[:], scalar1=0.5,
                            scalar2=None, op0=OP.mult)

    # ---- main loop over query tiles, grouped for the matmul phase ----
    # The cov fixup (PSUM acc -> SBUF -> PE transpose -> cov slab) for group g
    # is emitted two groups later so its PSUM reads never head-of-line block
    # the ACT/PE streams of the current group.
    assert T % GROUP == 0
    pend = {}

    def fixup(g):
        acc_g = pend.pop(g)
        covg = work.tile([NF, P * GROUP], F32, tag="covg", name="covg", bufs=1)
        nc.scalar.activation(covg[:], acc_g[:], AF.Identity,
                             bias=ftot_h[:], scale=0.5)
        for tt in range(GROUP):
            t = g * GROUP + tt
            ctp = psum.tile([P, NF], F32, tag="ctp", name="ctp", bufs=3)
            nc.tensor.matmul(ctp[:], lhsT=covg[:, tt * P:(tt + 1) * P],
                             rhs=ident18[0:NF, 0:NF], is_transpose=True)
            nc.scalar.activation(cov[:, t, :], ctp[:], AF.Copy, bias=0.0,
                                 scale=1.0)

    # mask emission for tile t is deferred into tile t+1 so the ACT Sign pass
    # never head-of-line blocks the next tile's Square passes.
    mask_q = []

    def emit_mask():
        negdm_p, m2_p, wt_p, tt_p = mask_q.pop(0)
        nt16p = small.tile([P, 1], F32, tag="nt16p", name="nt16p")
        nc.vector.tensor_scalar(out=nt16p[:], in0=m2_p[:, 7:8],
                                scalar1=-(1.0 + 2.0 ** -22), scalar2=None,
                                op0=OP.mult)
        wmask = work.tile([P, M], BF16, tag="wmask", name="wmask")
        sp = SPLIT * P
        nc.scalar.activation(wmask[:, 0:sp], negdm_p[:, 0:sp], AF.Sign,
                             bias=nt16p[:], scale=1.0)
        nc.vector.tensor_scalar(out=wmask[:, sp:M], in0=negdm_p[:, sp:M],
                                scalar1=m2_p[:, 7:8], scalar2=2.0,
                                op0=OP.is_ge, op1=OP.mult)
        nc.sync.dma_start(wt_p[:, :, tt_p * P:(tt_p + 1) * P], wmask[:],
                          transpose=True)

    def emit_matmuls(g, wt_g):
        acc = psum.tile([NF, P * GROUP], F32, tag="acc", name="acc", bufs=4)
        for j in range(T):
            nc.tensor.matmul(acc[:], lhsT=fbf[:, j, :], rhs=wt_g[:, j, :],
                             start=(j == 0), stop=(j == T - 1))
        pend[g] = acc

    prev_wt = None
    for g in range(T // GROUP):
        if g >= 2:
            fixup(g - 2)
        wt = work.tile([P, T, P * GROUP], BF16, tag="wt", name="wt", bufs=1)
        for tt in range(GROUP):
            t = g * GROUP + tt
            # exact f32 squared diffs: ACT Square with per-partition bias -u_q
            squ = work.tile([P, M], F32, tag="sq", name="squ", bufs=3)
            nc.scalar.activation(squ[:], u_b[:], AF.Square,
                                 bias=nuv[:, t, 0:1], scale=1.0)
            sqv = work.tile([P, M], F32, tag="sq", name="sqv", bufs=3)
            nc.scalar.activation(sqv[:], v_b[:], AF.Square,
                                 bias=nuv[:, t, 1:2], scale=1.0)
            # negate in place (exact), then negdm = (-squ) - sqv
            nc.scalar.activation(squ[:], squ[:], AF.Copy, bias=0.0, scale=-1.0)
            negdm = work.tile([P, M], F32, tag="negdm", name="negdm", bufs=3)
            nc.gpsimd.tensor_tensor(out=negdm[:], in0=squ[:], in1=sqv[:],
                                    op=OP.subtract)
            # top-16 threshold per row
            m1 = small.tile([P, 8], F32, tag="m1", name="m1")
            nc.vector.max(m1[:], negdm[:])
            mr = work.tile([P, M], F32, tag="mr", name="mr", bufs=2)
            nc.vector.match_replace(mr[:], m1[:], negdm[:], NEG_BIG)
            m2 = small.tile([P, 8], F32, tag="m2", name="m2")
            nc.vector.max(m2[:], mr[:])
            # queue this tile's mask (+-1 ACT Sign + x-bar transpose into wt)
            mask_q.append((negdm, m2, wt, tt))
            if len(mask_q) > 2:
                emit_mask()
        # matmuls for the previous group (its masks are all emitted by now)
        if prev_wt is not None:
            emit_matmuls(g - 1, prev_wt)
        prev_wt = wt

    while mask_q:
        emit_mask()
    emit_matmuls(T // GROUP - 1, prev_wt)
    for g in sorted(pend):
        fixup(g)

    # ---- eigen phase (slabs [P, T]) ----
    def et(name, shape=None):
        return epool.tile(shape or [P, T], F32, tag=name, name=name)

    vec = nc.vector

    def tt_(out, a, b, op):
        vec.tensor_tensor(out=out, in0=a, in1=b, op=op)

    S = et("S", [P, T, 9])
    tt_(S[:], cov[:, :, 0:9], cov[:, :, 9:18], OP.add)
    Sq = et("Sq", [P, T, 3])
    vec.tensor_scalar(out=Sq[:], in0=S[:, :, 0:3], scalar1=0.25, scalar2=None,
                      op0=OP.mult)
    cm = et("cm", [P, T, 6])
    tmp = et("tmp")
    for i, (a, b) in enumerate(pairs):
        tt_(tmp[:], Sq[:, :, a], Sq[:, :, b], OP.mult)
        tt_(cm[:, :, i], S[:, :, 3 + i], tmp[:], OP.subtract)

    cxx, cyy, czz = cm[:, :, 0], cm[:, :, 1], cm[:, :, 2]
    cxy, cxz, cyz = cm[:, :, 3], cm[:, :, 4], cm[:, :, 5]

    q = et("q")
    tt_(q[:], cxx, cyy, OP.add)
    tt_(q[:], q[:], czz, OP.add)
    vec.tensor_scalar(out=q[:], in0=q[:], scalar1=1.0 / 3.0, scalar2=None,
                      op0=OP.mult)
    b00, b11, b22 = et("b00"), et("b11"), et("b22")
    tt_(b00[:], cxx, q[:], OP.subtract)
    tt_(b11[:], cyy, q[:], OP.subtract)
    tt_(b22[:], czz, q[:], OP.subtract)
    p2 = et("p2")
    ta, tb = et("ta"), et("tb")
    tt_(p2[:], b00[:], b00[:], OP.mult)
    tt_(ta[:], b11[:], b11[:], OP.mult)
    tt_(p2[:], p2[:], ta[:], OP.add)
    tt_(ta[:], b22[:], b22[:], OP.mult)
    tt_(p2[:], p2[:], ta[:], OP.add)
    tt_(ta[:], cxy, cxy, OP.mult)
    tt_(tb[:], cxz, cxz, OP.mult)
    tt_(ta[:], ta[:], tb[:], OP.add)
    tt_(tb[:], cyz, cyz, OP.mult)
    tt_(ta[:], ta[:], tb[:], OP.add)
    vec.tensor_scalar(out=ta[:], in0=ta[:], scalar1=2.0, scalar2=None,
                      op0=OP.mult)
    tt_(p2[:], p2[:], ta[:], OP.add)
    p = et("p")
    nc.scalar.activation(p[:], p2[:], AF.Sqrt, bias=0.0, scale=1.0 / 6.0)
    pc = et("pc")
    vec.tensor_scalar(out=pc[:], in0=p[:], scalar1=1e-30, scalar2=None,
                      op0=OP.max)
    ip = et("ip")
    vec.reciprocal(ip[:], pc[:])
    p2x = et("p2x")
    vec.tensor_scalar(out=p2x[:], in0=p[:], scalar1=2.0, scalar2=None,
                      op0=OP.mult)
    # det(A - qI)
    det = et("det")
    tt_(ta[:], b11[:], b22[:], OP.mult)
    tt_(tb[:], cyz, cyz, OP.mult)
    tt_(ta[:], ta[:], tb[:], OP.subtract)
    tt_(det[:], b00[:], ta[:], OP.mult)
    tt_(ta[:], cxy, b22[:], OP.mult)
    tt_(tb[:], cyz, cxz, OP.mult)
    tt_(ta[:], ta[:], tb[:], OP.subtract)
    tt_(ta[:], cxy, ta[:], OP.mult)
    tt_(det[:], det[:], ta[:], OP.subtract)
    tt_(ta[:], cxy, cyz, OP.mult)
    tt_(tb[:], b11[:], cxz, OP.mult)
    tt_(ta[:], ta[:], tb[:], OP.subtract)
    tt_(ta[:], cxz, ta[:], OP.mult)
    tt_(det[:], det[:], ta[:], OP.add)
    # r = clamp(det * ip^3 / 2, -1, 1)
    r = et("r")
    tt_(ta[:], ip[:], ip[:], OP.mult)
    tt_(ta[:], ta[:], ip[:], OP.mult)
    tt_(r[:], det[:], ta[:], OP.mult)
    vec.tensor_scalar(out=r[:], in0=r[:], scalar1=0.5, scalar2=1.0,
                      op0=OP.mult, op1=OP.min)
    vec.tensor_scalar(out=r[:], in0=r[:], scalar1=-1.0, scalar2=None,
                      op0=OP.max)
    # acos(r) via octant-reduced arctan
    rr = et("rr")
    tt_(rr[:], r[:], r[:], OP.mult)
    aab = et("aab")
    nc.scalar.activation(aab[:], rr[:], AF.Sqrt, bias=0.0, scale=1.0)  # |r|
    vec.tensor_scalar(out=rr[:], in0=rr[:], scalar1=-1.0, scalar2=1.0,
                      op0=OP.mult, op1=OP.add)
    s = et("s")
    nc.scalar.activation(s[:], rr[:], AF.Sqrt, bias=0.0, scale=1.0)
    mn, mx = et("mn"), et("mx")
    tt_(mn[:], aab[:], s[:], OP.min)
    tt_(mx[:], aab[:], s[:], OP.max)
    imx = et("imx")
    vec.reciprocal(imx[:], mx[:])
    ratio = et("ratio")
    tt_(ratio[:], mn[:], imx[:], OP.mult)
    th = et("th")
    nc.scalar.activation(th[:], ratio[:], AF.Arctan, bias=0.0, scale=1.0)
    mk = et("mk")
    tt_(mk[:], s[:], aab[:], OP.is_gt)
    u1 = et("u1")
    vec.tensor_scalar(out=u1[:], in0=th[:], scalar1=-2.0, scalar2=PI / 2,
                      op0=OP.mult, op1=OP.add)
    tt_(u1[:], mk[:], u1[:], OP.mult)
    tt_(th[:], th[:], u1[:], OP.add)
    vec.tensor_scalar(out=mk[:], in0=r[:], scalar1=0.0, scalar2=None,
                      op0=OP.is_lt)
    vec.tensor_scalar(out=u1[:], in0=th[:], scalar1=-2.0, scalar2=PI,
                      op0=OP.mult, op1=OP.add)
    tt_(u1[:], mk[:], u1[:], OP.mult)
    tt_(th[:], th[:], u1[:], OP.add)
    phi = et("phi")
    vec.tensor_scalar(out=phi[:], in0=th[:], scalar1=1.0 / 3.0, scalar2=None,
                      op0=OP.mult)
    # cos(phi) = sin(phi + pi/2);  cos(phi + 2pi/3) = -sin(phi + pi/6)
    bias_c = et("bias_c", [P, 2])
    nc.gpsimd.memset(bias_c[:, 0:1], PI / 2)
    nc.gpsimd.memset(bias_c[:, 1:2], PI / 6)
    c1, c3 = et("c1"), et("c3")
    nc.scalar.activation(c1[:], phi[:], AF.Sin, bias=bias_c[:, 0:1], scale=1.0)
    nc.scalar.activation(c3[:], phi[:], AF.Sin, bias=bias_c[:, 1:2],
                         scale=1.0)
    eigs = et("eigs", [P, T, 3])
    tt_(ta[:], p2x[:], c1[:], OP.mult)
    tt_(eigs[:, :, 0], ta[:], q[:], OP.add)       # e1 = q + 2p*cos(phi)
    tt_(tb[:], p2x[:], c3[:], OP.mult)
    tt_(eigs[:, :, 2], q[:], tb[:], OP.subtract)  # e3 = q - 2p*sin(phi+pi/6)
    q3 = et("q3")
    vec.tensor_scalar(out=q3[:], in0=q[:], scalar1=3.0, scalar2=None,
                      op0=OP.mult)
    tt_(q3[:], q3[:], eigs[:, :, 0], OP.subtract)
    tt_(eigs[:, :, 1], q3[:], eigs[:, :, 2], OP.subtract)  # e2 = 3q - e1 - e3

    out_r = out_ap.rearrange("(t p) k -> p t k", p=P)
    for d in range(NDMA):
        sl = slice(d * tchunk, (d + 1) * tchunk)
        nc.sync.dma_start(out_r[:, sl, :], eigs[:, sl, :])


def _emit_with_ident(ctx, tc, out_ap, x_ap, uv_ap, M):
    # identity matrix (bf16) for the PE cov-transpose, built once
    nc = tc.nc
    const = ctx.enter_context(tc.tile_pool(name="identc", bufs=1))
    iota_a = const.tile([P, P], I32, tag="iota_a", name="iota_a")
    nc.gpsimd.iota(iota_a[:], pattern=[[1, P]], base=0, channel_multiplier=0)
    iota_b = const.tile([P, 1], I32, tag="iota_b", name="iota_b")
    nc.gpsimd.iota(iota_b[:], pattern=[[1, 1]], base=0, channel_multiplier=1)
    iota_af = const.tile([P, P], F32, tag="iota_af", name="iota_af")
    nc.gpsimd.tensor_copy(iota_af[:], iota_a[:])
    iota_bf = const.tile([P, 1], F32, tag="iota_bf", name="iota_bf")
    nc.gpsimd.tensor_copy(iota_bf[:], iota_b[:])
    ident = const.tile([P, P], F32, tag="ident", name="ident")
    nc.gpsimd.tensor_scalar(out=ident[:], in0=iota_af[:],
                            scalar1=iota_bf[:, 0:1],
                            scalar2=None, op0=OP.is_equal)
    _emit(ctx, tc, out_ap, x_ap, uv_ap, M, ident)


def build_nc(M: int = 4096):
    nc = bacc.Bacc("TRN2", target_bir_lowering=False, debug=False,
                   enable_asserts=False)
    x_ap = nc.dram_tensor("X", (M, 3), F32, kind="ExternalInput").ap()
    uv_ap = nc.dram_tensor("uv", (M, 2), F32, kind="ExternalInput").ap()
    out_ap = nc.dram_tensor("out", (M, 3), F32, kind="ExternalOutput").ap()
    with tile.TileContext(nc) as tc:
        with ExitStack() as ctx:
            _emit_with_ident(ctx, tc, out_ap, x_ap, uv_ap, M)
    nc.compile()
    return nc


_NC_CACHE = {}


def _get_nc(M: int = 4096):
    if M not in _NC_CACHE:
        _NC_CACHE[M] = build_nc(M)
    return _NC_CACHE[M]


def run(X, uv, trace: bool = False):
    B, M, _ = X.shape
    nc = _get_nc(M)
    in_maps = [
        {"X": np.ascontiguousarray(X[b], dtype=np.float32),
         "uv": np.ascontiguousarray(uv[b], dtype=np.float32)}
        for b in range(B)
    ]
    res = run_bass_kernel_spmd(nc, in_maps, core_ids=list(range(B)),
                               trace=trace)
    out = np.stack([r["out"] for r in res.results], axis=0)
    return out, res


def kernel(X, uv):
    X = np.asarray(X)
    uv = np.asarray(uv)
    out, _ = run(X, uv, trace=False)
    return out.astype(np.float32)


# revision 36
# speedup vs baseline: 1.0690x; 1.0690x over previous
"""Trainium2 Bass kernel for DiffGeomPropsApprox (within-batch uv-space 16-NN
-> neighborhood covariance of X -> descending symmetric-3x3 eigenvalues).

Sharding: data-parallel over batch B=8, one batch per NeuronCore (8 cores).

Per-core algorithm (M=4096 points):
  * negdm[i,j] = -((u_i-u_j)^2 + (v_i-v_j)^2), computed with the exact same
    f32 roundings as the reference (sub -> square -> add, negation is exact),
    so the NN ranking is bit-identical to the reference's top_k.
  * per query row: DVE max8 -> match_replace -> max8 gives t16 = the 16th
    largest negdm value; the selection mask is w = (negdm >= t16) in bf16.
  * mask transposed via DMA x-bar transpose (idle DMA engines), cov sums via
    PE: feature-stationary matmuls (18 features = 9 cols x bf16 hi/lo; the
    0/1 mask is exact in bf16) accumulated in PSUM as [18, q].
  * cov = S2 - outer(S1)/16; closed-form symmetric 3x3 eigenvalues
    (trigonometric method; acos/cos built from Arctan/Sin/Sqrt LUTs).
"""

from contextlib import ExitStack

import numpy as np

import concourse.bass as bass
import concourse.tile as tile
from concourse import bacc, mybir
from concourse.alu_op_type import AluOpType
from concourse.bass_utils import run_bass_kernel_spmd

F32 = mybir.dt.float32
BF16 = mybir.dt.bfloat16
I32 = mybir.dt.int32
AF = mybir.ActivationFunctionType
OP = AluOpType

P = 128
K = 16
GROUP = 2           # query tiles per matmul group
NEG_BIG = -3.0e38
PI = float(np.pi)


def _emit(ctx: ExitStack, tc, out_ap, x_ap, uv_ap, M: int, ident18):
    nc = tc.nc
    T = M // P          # number of 128-row tiles
    NF = 18             # 9 features x (hi, lo)
    NDMA = min(8, T)    # split slow strided DMAs this many ways

    const = ctx.enter_context(tc.tile_pool(name="const", bufs=1))
    work = ctx.enter_context(tc.tile_pool(name="work", bufs=2))
    small = ctx.enter_context(tc.tile_pool(name="small", bufs=4))
    psum = ctx.enter_context(tc.tile_pool(name="psum", bufs=2, space="PSUM"))
    epool = ctx.enter_context(tc.tile_pool(name="eig", bufs=1))

    # ---- broadcast candidate coords across partitions (doubling DMAs) ----
    u_b = const.tile([P, M], F32, tag="u_b")
    v_b = const.tile([P, M], F32, tag="v_b")
    uv_t = uv_ap.rearrange("m k -> k m")
    nc.sync.dma_start(u_b[0:1, :], uv_t[0:1, :])
    nc.sync.dma_start(v_b[0:1, :], uv_t[1:2, :])
    k = 1
    while k < P:
        nc.sync.dma_start(u_b[k:2 * k, :], u_b[0:k, :])
        nc.sync.dma_start(v_b[k:2 * k, :], v_b[0:k, :])
        k *= 2

    # slab layouts [P, T, k]: point m = t*128 + p  (strided loads, split)
    uv_slab = const.tile([P, T, 2], F32, tag="uv_slab")
    uv_r = uv_ap.rearrange("(t p) k -> p t k", p=P)
    x_slab = const.tile([P, T, 3], F32, tag="x_slab")
    x_r = x_ap.rearrange("(t p) k -> p t k", p=P)
    tchunk = T // NDMA
    for d in range(NDMA):
        sl = slice(d * tchunk, (d + 1) * tchunk)
        nc.sync.dma_start(uv_slab[:, sl, :], uv_r[:, sl, :])
        nc.sync.dma_start(x_slab[:, sl, :], x_r[:, sl, :])
    nuv = const.tile([P, T, 2], F32, tag="nuv")
    nc.vector.tensor_scalar(out=nuv[:], in0=uv_slab[:], scalar1=-1.0,
                            scalar2=None, op0=OP.mult)

    # ---- features: [x y z x2 y2 z2 xy xz yz] as bf16 hi/lo ----
    pairs = [(0, 0), (1, 1), (2, 2), (0, 1), (0, 2), (1, 2)]
    fsl = work.tile([P, T, 9], F32, tag="fsl", name="fsl", bufs=1)
    nc.vector.tensor_copy(fsl[:, :, 0:3], x_slab[:])
    for i, (a, b) in enumerate(pairs):
        nc.vector.tensor_tensor(out=fsl[:, :, 3 + i], in0=x_slab[:, :, a],
                                in1=x_slab[:, :, b], op=OP.mult)
    fbf = const.tile([P, T, NF], BF16, tag="fbf")
    nc.vector.tensor_copy(fbf[:, :, 0:9], fsl[:])
    fhi32 = work.tile([P, T, 9], F32, tag="fhi32", name="fhi32", bufs=1)
    nc.vector.tensor_copy(fhi32[:], fbf[:, :, 0:9])
    nc.vector.tensor_tensor(out=fbf[:, :, 9:18], in0=fsl[:], in1=fhi32[:],
                            op=OP.subtract)

    cov = const.tile([P, T, NF], F32, tag="cov")

    # per-feature totals sum_c F[c, f] (for the +-1 sign-mask trick):
    # acc = sum_sel f - sum_unsel f  =>  sum_sel = (acc + ftot) / 2
    ones_c = const.tile([P, 1], BF16, tag="ones_c")
    nc.gpsimd.memset(ones_c[:], 1.0)
    ftot_ps = psum.tile([NF, 1], F32, tag="ftot", name="ftot_ps", bufs=1)
    for j in range(T):
        nc.tensor.matmul(ftot_ps[:], lhsT=fbf[:, j, :], rhs=ones_c[:],
                         start=(j == 0), stop=(j == T - 1))
    ftot_h = const.tile([NF, 1], F32, tag="ftot_h")
    nc.vector.tensor_scalar(out=ftot_h[:], in0=ftot_ps[:], scalar1=0.5,
                            scalar2=None, op0=OP.mult)

    # ---- main loop over query tiles, grouped for the matmul phase ----
    # The cov fixup (PSUM acc -> SBUF -> PE transpose -> cov slab) for group g
    # is emitted two groups later so its PSUM reads never head-of-line block
    # the ACT/PE streams of the current group.
    assert T % GROUP == 0
    pend = {}

    def fixup(g):
        acc_g = pend.pop(g)
        covg = work.tile([NF, P * GROUP], F32, tag="covg", name="covg", bufs=1)
        nc.scalar.activation(covg[:], acc_g[:], AF.Identity,
                             bias=ftot_h[:], scale=0.5)
        for tt in range(GROUP):
            t = g * GROUP + tt
            ctp = psum.tile([P, NF], F32, tag="ctp", name="ctp", bufs=3)
            nc.tensor.matmul(ctp[:], lhsT=covg[:, tt * P:(tt + 1) * P],
                             rhs=ident18[0:NF, 0:NF], is_transpose=True)
            nc.scalar.activation(cov[:, t, :], ctp[:], AF.Copy, bias=0.0,
                                 scale=1.0)

    # mask emission for tile t is deferred into tile t+1 so the ACT Sign pass
    # never head-of-line blocks the next tile's Square passes.
    mask_q = []

    def emit_mask():
        negdm_p, m2_p, wt_p, tt_p = mask_q.pop(0)
        nt16p = small.tile([P, 1], F32, tag="nt16p", name="nt16p")
        nc.vector.tensor_scalar(out=nt16p[:], in0=m2_p[:, 7:8],
                                scalar1=-(1.0 + 2.0 ** -22), scalar2=None,
                                op0=OP.mult)
        wmask = work.tile([P, M], BF16, tag="wmask", name="wmask")
        nc.scalar.activation(wmask[:], negdm_p[:], AF.Sign,
                             bias=nt16p[:], scale=1.0)
        nc.sync.dma_start(wt_p[:, :, tt_p * P:(tt_p + 1) * P], wmask[:],
                          transpose=True)

    def emit_matmuls(g, wt_g):
        acc = psum.tile([NF, P * GROUP], F32, tag="acc", name="acc", bufs=4)
        for j in range(T):
            nc.tensor.matmul(acc[:], lhsT=fbf[:, j, :], rhs=wt_g[:, j, :],
                             start=(j == 0), stop=(j == T - 1))
        pend[g] = acc

    prev_wt = None
    for g in range(T // GROUP):
        if g >= 2:
            fixup(g - 2)
        wt = work.tile([P, T, P * GROUP], BF16, tag="wt", name="wt", bufs=1)
        for tt in range(GROUP):
            t = g * GROUP + tt
            # exact f32 squared diffs: ACT Square with per-partition bias -u_q
            squ = work.tile([P, M], F32, tag="sq", name="squ", bufs=3)
            nc.scalar.activation(squ[:], u_b[:], AF.Square,
                                 bias=nuv[:, t, 0:1], scale=1.0)
            sqv = work.tile([P, M], F32, tag="sq", name="sqv", bufs=3)
            nc.scalar.activation(sqv[:], v_b[:], AF.Square,
                                 bias=nuv[:, t, 1:2], scale=1.0)
            # negate in place (exact), then negdm = (-squ) - sqv
            nc.scalar.activation(squ[:], squ[:], AF.Copy, bias=0.0, scale=-1.0)
            negdm = work.tile([P, M], F32, tag="negdm", name="negdm", bufs=3)
            nc.gpsimd.tensor_tensor(out=negdm[:], in0=squ[:], in1=sqv[:],
                                    op=OP.subtract)
            # top-16 threshold per row
            m1 = small.tile([P, 8], F32, tag="m1", name="m1")
            nc.vector.max(m1[:], negdm[:])
            mr = work.tile([P, M], F32, tag="mr", name="mr", bufs=2)
            nc.vector.match_replace(mr[:], m1[:], negdm[:], NEG_BIG)
            m2 = small.tile([P, 8], F32, tag="m2", name="m2")
            nc.vector.max(m2[:], mr[:])
            # queue this tile's mask (+-1 ACT Sign + x-bar transpose into wt)
            mask_q.append((negdm, m2, wt, tt))
            if len(mask_q) > 2:
                emit_mask()
        # matmuls for the previous group (its masks are all emitted by now)
        if prev_wt is not None:
            emit_matmuls(g - 1, prev_wt)
        prev_wt = wt

    while mask_q:
        emit_mask()
    emit_matmuls(T // GROUP - 1, prev_wt)
    for g in sorted(pend):
        fixup(g)

    # ---- eigen phase (slabs [P, T]) ----
    def et(name, shape=None):
        return epool.tile(shape or [P, T], F32, tag=name, name=name)

    vec = nc.vector

    def tt_(out, a, b, op):
        vec.tensor_tensor(out=out, in0=a, in1=b, op=op)

    S = et("S", [P, T, 9])
    tt_(S[:], cov[:, :, 0:9], cov[:, :, 9:18], OP.add)
    Sq = et("Sq", [P, T, 3])
    vec.tensor_scalar(out=Sq[:], in0=S[:, :, 0:3], scalar1=0.25, scalar2=None,
                      op0=OP.mult)
    cm = et("cm", [P, T, 6])
    tmp = et("tmp")
    for i, (a, b) in enumerate(pairs):
        tt_(tmp[:], Sq[:, :, a], Sq[:, :, b], OP.mult)
        tt_(cm[:, :, i], S[:, :, 3 + i], tmp[:], OP.subtract)

    cxx, cyy, czz = cm[:, :, 0], cm[:, :, 1], cm[:, :, 2]
    cxy, cxz, cyz = cm[:, :, 3], cm[:, :, 4], cm[:, :, 5]

    q = et("q")
    tt_(q[:], cxx, cyy, OP.add)
    tt_(q[:], q[:], czz, OP.add)
    vec.tensor_scalar(out=q[:], in0=q[:], scalar1=1.0 / 3.0, scalar2=None,
                      op0=OP.mult)
    b00, b11, b22 = et("b00"), et("b11"), et("b22")
    tt_(b00[:], cxx, q[:], OP.subtract)
    tt_(b11[:], cyy, q[:], OP.subtract)
    tt_(b22[:], czz, q[:], OP.subtract)
    p2 = et("p2")
    ta, tb = et("ta"), et("tb")
    tt_(p2[:], b00[:], b00[:], OP.mult)
    tt_(ta[:], b11[:], b11[:], OP.mult)
    tt_(p2[:], p2[:], ta[:], OP.add)
    tt_(ta[:], b22[:], b22[:], OP.mult)
    tt_(p2[:], p2[:], ta[:], OP.add)
    tt_(ta[:], cxy, cxy, OP.mult)
    tt_(tb[:], cxz, cxz, OP.mult)
    tt_(ta[:], ta[:], tb[:], OP.add)
    tt_(tb[:], cyz, cyz, OP.mult)
    tt_(ta[:], ta[:], tb[:], OP.add)
    vec.tensor_scalar(out=ta[:], in0=ta[:], scalar1=2.0, scalar2=None,
                      op0=OP.mult)
    tt_(p2[:], p2[:], ta[:], OP.add)
    p = et("p")
    nc.scalar.activation(p[:], p2[:], AF.Sqrt, bias=0.0, scale=1.0 / 6.0)
    pc = et("pc")
    vec.tensor_scalar(out=pc[:], in0=p[:], scalar1=1e-30, scalar2=None,
                      op0=OP.max)
    ip = et("ip")
    vec.reciprocal(ip[:], pc[:])
    p2x = et("p2x")
    vec.tensor_scalar(out=p2x[:], in0=p[:], scalar1=2.0, scalar2=None,
                      op0=OP.mult)
    # det(A - qI)
    det = et("det")
    tt_(ta[:], b11[:], b22[:], OP.mult)
    tt_(tb[:], cyz, cyz, OP.mult)
    tt_(ta[:], ta[:], tb[:], OP.subtract)
    tt_(det[:], b00[:], ta[:], OP.mult)
    tt_(ta[:], cxy, b22[:], OP.mult)
    tt_(tb[:], cyz, cxz, OP.mult)
    tt_(ta[:], ta[:], tb[:], OP.subtract)
    tt_(ta[:], cxy, ta[:], OP.mult)
    tt_(det[:], det[:], ta[:], OP.subtract)
    tt_(ta[:], cxy, cyz, OP.mult)
    tt_(tb[:], b11[:], cxz, OP.mult)
    tt_(ta[:], ta[:], tb[:], OP.subtract)
    tt_(ta[:], cxz, ta[:], OP.mult)
    tt_(det[:], det[:], ta[:], OP.add)
    # r = clamp(det * ip^3 / 2, -1, 1)
    r = et("r")
    tt_(ta[:], ip[:], ip[:], OP.mult)
    tt_(ta[:], ta[:], ip[:], OP.mult)
    tt_(r[:], det[:], ta[:], OP.mult)
    vec.tensor_scalar(out=r[:], in0=r[:], scalar1=0.5, scalar2=1.0,
                      op0=OP.mult, op1=OP.min)
    vec.tensor_scalar(out=r[:], in0=r[:], scalar1=-1.0, scalar2=None,
                      op0=OP.max)
    # acos(r) via octant-reduced arctan
    rr = et("rr")
    tt_(rr[:], r[:], r[:], OP.mult)
    aab = et("aab")
    nc.scalar.activation(aab[:], rr[:], AF.Sqrt, bias=0.0, scale=1.0)  # |r|
    vec.tensor_scalar(out=rr[:], in0=rr[:], scalar1=-1.0, scalar2=1.0,
                      op0=OP.mult, op1=OP.add)
    s = et("s")
    nc.scalar.activation(s[:], rr[:], AF.Sqrt, bias=0.0, scale=1.0)
    mn, mx = et("mn"), et("mx")
    tt_(mn[:], aab[:], s[:], OP.min)
    tt_(mx[:], aab[:], s[:], OP.max)
    imx = et("imx")
    vec.reciprocal(imx[:], mx[:])
    ratio = et("ratio")
    tt_(ratio[:], mn[:], imx[:], OP.mult)
    th = et("th")
    nc.scalar.activation(th[:], ratio[:], AF.Arctan, bias=0.0, scale=1.0)
    mk = et("mk")
    tt_(mk[:], s[:], aab[:], OP.is_gt)
    u1 = et("u1")
    vec.tensor_scalar(out=u1[:], in0=th[:], scalar1=-2.0, scalar2=PI / 2,
                      op0=OP.mult, op1=OP.add)
    tt_(u1[:], mk[:], u1[:], OP.mult)
    tt_(th[:], th[:], u1[:], OP.add)
    vec.tensor_scalar(out=mk[:], in0=r[:], scalar1=0.0, scalar2=None,
                      op0=OP.is_lt)
    vec.tensor_scalar(out=u1[:], in0=th[:], scalar1=-2.0, scalar2=PI,
                      op0=OP.mult, op1=OP.add)
    tt_(u1[:], mk[:], u1[:], OP.mult)
    tt_(th[:], th[:], u1[:], OP.add)
    phi = et("phi")
    vec.tensor_scalar(out=phi[:], in0=th[:], scalar1=1.0 / 3.0, scalar2=None,
                      op0=OP.mult)
    # cos(phi) = sin(phi + pi/2);  cos(phi + 2pi/3) = -sin(phi + pi/6)
    bias_c = et("bias_c", [P, 2])
    nc.gpsimd.memset(bias_c[:, 0:1], PI / 2)
    nc.gpsimd.memset(bias_c[:, 1:2], PI / 6)
    c1, c3 = et("c1"), et("c3")
    nc.scalar.activation(c1[:], phi[:], AF.Sin, bias=bias_c[:, 0:1], scale=1.0)
    nc.scalar.activation(c3[:], phi[:], AF.Sin, bias=bias_c[:, 1:2],
                         scale=1.0)
    eigs = et("eigs", [P, T, 3])
    tt_(ta[:], p2x[:], c1[:], OP.mult)
    tt_(eigs[:, :, 0], ta[:], q[:], OP.add)       # e1 = q + 2p*cos(phi)
    tt_(tb[:], p2x[:], c3[:], OP.mult)
    tt_(eigs[:, :, 2], q[:], tb[:], OP.subtract)  # e3 = q - 2p*sin(phi+pi/6)
    q3 = et("q3")
    vec.tensor_scalar(out=q3[:], in0=q[:], scalar1=3.0, scalar2=None,
                      op0=OP.mult)
    tt_(q3[:], q3[:], eigs[:, :, 0], OP.subtract)
    tt_(eigs[:, :, 1], q3[:], eigs[:, :, 2], OP.subtract)  # e2 = 3q - e1 - e3

    out_r = out_ap.rearrange("(t p) k -> p t k", p=P)
    for d in range(NDMA):
        sl = slice(d * tchunk, (d + 1) * tchunk)
        nc.sync.dma_start(out_r[:, sl, :], eigs[:, sl, :])


def _emit_with_ident(ctx, tc, out_ap, x_ap, uv_ap, M):
    # identity matrix (bf16) for the PE cov-transpose, built once
    nc = tc.nc
    const = ctx.enter_context(tc.tile_pool(name="identc", bufs=1))
    iota_a = const.tile([P, P], I32, tag="iota_a", name="iota_a")
    nc.gpsimd.iota(iota_a[:], pattern=[[1, P]], base=0, channel_multiplier=0)
    iota_b = const.tile([P, 1], I32, tag="iota_b", name="iota_b")
    nc.gpsimd.iota(iota_b[:], pattern=[[1, 1]], base=0, channel_multiplier=1)
    iota_af = const.tile([P, P], F32, tag="iota_af", name="iota_af")
    nc.gpsimd.tensor_copy(iota_af[:], iota_a[:])
    iota_bf = const.tile([P, 1], F32, tag="iota_bf", name="iota_bf")
    nc.gpsimd.tensor_copy(iota_bf[:], iota_b[:])
    ident = const.tile([P, P], F32, tag="ident", name="ident")
    nc.gpsimd.tensor_scalar(out=ident[:], in0=iota_af[:],
                            scalar1=iota_bf[:, 0:1],
                            scalar2=None, op0=OP.is_equal)
    _emit(ctx, tc, out_ap, x_ap, uv_ap, M, ident)


def build_nc(M: int = 4096):
    nc = bacc.Bacc("TRN2", target_bir_lowering=False, debug=False,
                   enable_asserts=False)
    x_ap = nc.dram_tensor("X", (M, 3), F32, kind="ExternalInput").ap()
    uv_ap = nc.dram_tensor("uv", (M, 2), F32, kind="ExternalInput").ap()
    out_ap = nc.dram_tensor("out", (M, 3), F32, kind="ExternalOutput").ap()
    with tile.TileContext(nc) as tc:
        with ExitStack() as ctx:
            _emit_with_ident(ctx, tc, out_ap, x_ap, uv_ap, M)
    nc.compile()
    return nc


_NC_CACHE = {}


def _get_nc(M: int = 4096):
    if M not in _NC_CACHE:
        _NC_CACHE[M] = build_nc(M)
    return _NC_CACHE[M]


def run(X, uv, trace: bool = False):
    B, M, _ = X.shape
    nc = _get_nc(M)
    in_maps = [
        {"X": np.ascontiguousarray(X[b], dtype=np.float32),
         "uv": np.ascontiguousarray(uv[b], dtype=np.float32)}
        for b in range(B)
    ]
    res = run_bass_kernel_spmd(nc, in_maps, core_ids=list(range(B)),
                               trace=trace)
    out = np.stack([r["out"] for r in res.results], axis=0)
    return out, res


def kernel(X, uv):
    X = np.asarray(X)
    uv = np.asarray(uv)
    out, _ = run(X, uv, trace=False)
    return out.astype(np.float32)


# revision 39
# speedup vs baseline: 1.0743x; 1.0050x over previous
"""Trainium2 Bass kernel for DiffGeomPropsApprox (within-batch uv-space 16-NN
-> neighborhood covariance of X -> descending symmetric-3x3 eigenvalues).

Sharding: data-parallel over batch B=8, one batch per NeuronCore (8 cores).

Per-core algorithm (M=4096 points):
  * negdm[i,j] = -((u_i-u_j)^2 + (v_i-v_j)^2), computed with the exact same
    f32 roundings as the reference (sub -> square -> add, negation is exact),
    so the NN ranking is bit-identical to the reference's top_k.
  * per query row: DVE max8 -> match_replace -> max8 gives t16 = the 16th
    largest negdm value; the selection mask is w = (negdm >= t16) in bf16.
  * mask transposed via DMA x-bar transpose (idle DMA engines), cov sums via
    PE: feature-stationary matmuls (18 features = 9 cols x bf16 hi/lo; the
    0/1 mask is exact in bf16) accumulated in PSUM as [18, q].
  * cov = S2 - outer(S1)/16; closed-form symmetric 3x3 eigenvalues
    (trigonometric method; acos/cos built from Arctan/Sin/Sqrt LUTs).
"""

from contextlib import ExitStack

import numpy as np

import concourse.bass as bass
import concourse.tile as tile
from concourse import bacc, mybir
from concourse.alu_op_type import AluOpType
from concourse.bass_utils import run_bass_kernel_spmd

F32 = mybir.dt.float32
BF16 = mybir.dt.bfloat16
I32 = mybir.dt.int32
AF = mybir.ActivationFunctionType
OP = AluOpType

P = 128
K = 16
GROUP = 2           # query tiles per matmul group
NEG_BIG = -3.0e38
PI = float(np.pi)


def _emit(ctx: ExitStack, tc, out_ap, x_ap, uv_ap, M: int, ident18):
    nc = tc.nc
    T = M // P          # number of 128-row tiles
    NF = 18             # 9 features x (hi, lo)
    NDMA = min(8, T)    # split slow strided DMAs this many ways

    const = ctx.enter_context(tc.tile_pool(name="const", bufs=1))
    work = ctx.enter_context(tc.tile_pool(name="work", bufs=2))
    small = ctx.enter_context(tc.tile_pool(name="small", bufs=4))
    psum = ctx.enter_context(tc.tile_pool(name="psum", bufs=2, space="PSUM"))
    epool = ctx.enter_context(tc.tile_pool(name="eig", bufs=1))

    # ---- broadcast candidate coords across partitions (doubling DMAs) ----
    u_b = const.tile([P, M], F32, tag="u_b")
    v_b = const.tile([P, M], F32, tag="v_b")
    uv_t = uv_ap.rearrange("m k -> k m")
    nc.sync.dma_start(u_b[0:1, :], uv_t[0:1, :])
    nc.sync.dma_start(v_b[0:1, :], uv_t[1:2, :])
    k = 1
    while k < P:
        nc.sync.dma_start(u_b[k:2 * k, :], u_b[0:k, :])
        nc.sync.dma_start(v_b[k:2 * k, :], v_b[0:k, :])
        k *= 2

    # slab layouts [P, T, k]: point m = t*128 + p  (strided loads, split)
    uv_slab = const.tile([P, T, 2], F32, tag="uv_slab")
    uv_r = uv_ap.rearrange("(t p) k -> p t k", p=P)
    x_slab = const.tile([P, T, 3], F32, tag="x_slab")
    x_r = x_ap.rearrange("(t p) k -> p t k", p=P)
    tchunk = T // NDMA
    for d in range(NDMA):
        sl = slice(d * tchunk, (d + 1) * tchunk)
        nc.sync.dma_start(uv_slab[:, sl, :], uv_r[:, sl, :])
        nc.sync.dma_start(x_slab[:, sl, :], x_r[:, sl, :])
    nuv = const.tile([P, T, 2], F32, tag="nuv")
    nc.vector.tensor_scalar(out=nuv[:], in0=uv_slab[:], scalar1=-1.0,
                            scalar2=None, op0=OP.mult)

    # ---- features: [x y z x2 y2 z2 xy xz yz] as bf16 hi/lo ----
    pairs = [(0, 0), (1, 1), (2, 2), (0, 1), (0, 2), (1, 2)]
    fsl = work.tile([P, T, 9], F32, tag="fsl", name="fsl", bufs=1)
    nc.vector.tensor_copy(fsl[:, :, 0:3], x_slab[:])
    for i, (a, b) in enumerate(pairs):
        nc.vector.tensor_tensor(out=fsl[:, :, 3 + i], in0=x_slab[:, :, a],
                                in1=x_slab[:, :, b], op=OP.mult)
    fbf = const.tile([P, T, NF], BF16, tag="fbf")
    nc.vector.tensor_copy(fbf[:, :, 0:9], fsl[:])
    fhi32 = work.tile([P, T, 9], F32, tag="fhi32", name="fhi32", bufs=1)
    nc.vector.tensor_copy(fhi32[:], fbf[:, :, 0:9])
    nc.vector.tensor_tensor(out=fbf[:, :, 9:18], in0=fsl[:], in1=fhi32[:],
                            op=OP.subtract)

    cov = const.tile([P, T, NF], F32, tag="cov")

    # per-feature totals sum_c F[c, f] (for the +-1 sign-mask trick):
    # acc = sum_sel f - sum_unsel f  =>  sum_sel = (acc + ftot) / 2
    ones_c = const.tile([P, 1], BF16, tag="ones_c")
    nc.gpsimd.memset(ones_c[:], 1.0)
    ftot_ps = psum.tile([NF, 1], F32, tag="ftot", name="ftot_ps", bufs=1)
    for j in range(T):
        nc.tensor.matmul(ftot_ps[:], lhsT=fbf[:, j, :], rhs=ones_c[:],
                         start=(j == 0), stop=(j == T - 1))
    ftot_h = const.tile([NF, 1], F32, tag="ftot_h")
    nc.vector.tensor_scalar(out=ftot_h[:], in0=ftot_ps[:], scalar1=0.5,
                            scalar2=None, op0=OP.mult)

    # ---- main loop over query tiles, grouped for the matmul phase ----
    # The cov fixup (PSUM acc -> SBUF -> PE transpose -> cov slab) for group g
    # is emitted two groups later so its PSUM reads never head-of-line block
    # the ACT/PE streams of the current group.
    assert T % GROUP == 0
    pend = {}

    def fixup(g):
        acc_g = pend.pop(g)
        covg = work.tile([NF, P * GROUP], F32, tag="covg", name="covg", bufs=1)
        nc.vector.tensor_scalar(out=covg[:], in0=acc_g[:], scalar1=0.5,
                                scalar2=ftot_h[:], op0=OP.mult, op1=OP.add)
        for tt in range(GROUP):
            t = g * GROUP + tt
            ctp = psum.tile([P, NF], F32, tag="ctp", name="ctp", bufs=3)
            nc.tensor.matmul(ctp[:], lhsT=covg[:, tt * P:(tt + 1) * P],
                             rhs=ident18[0:NF, 0:NF], is_transpose=True)
            nc.vector.tensor_copy(cov[:, t, :], ctp[:])

    # mask emission for tile t is deferred into tile t+1 so the ACT Sign pass
    # never head-of-line blocks the next tile's Square passes.
    mask_q = []

    def emit_mask():
        negdm_p, m2_p, wt_p, tt_p = mask_q.pop(0)
        nt16p = small.tile([P, 1], F32, tag="nt16p", name="nt16p")
        nc.vector.tensor_scalar(out=nt16p[:], in0=m2_p[:, 7:8],
                                scalar1=-(1.0 + 2.0 ** -22), scalar2=None,
                                op0=OP.mult)
        wmask = work.tile([P, M], BF16, tag="wmask", name="wmask")
        nc.scalar.activation(wmask[:], negdm_p[:], AF.Sign,
                             bias=nt16p[:], scale=1.0)
        nc.sync.dma_start(wt_p[:, :, tt_p * P:(tt_p + 1) * P], wmask[:],
                          transpose=True)

    def emit_matmuls(g, wt_g):
        acc = psum.tile([NF, P * GROUP], F32, tag="acc", name="acc", bufs=4)
        for j in range(T):
            nc.tensor.matmul(acc[:], lhsT=fbf[:, j, :], rhs=wt_g[:, j, :],
                             start=(j == 0), stop=(j == T - 1))
        pend[g] = acc

    prev_wt = None
    for g in range(T // GROUP):
        if g >= 2:
            fixup(g - 2)
        wt = work.tile([P, T, P * GROUP], BF16, tag="wt", name="wt", bufs=1)
        for tt in range(GROUP):
            t = g * GROUP + tt
            # exact f32 squared diffs: ACT Square with per-partition bias -u_q
            squ = work.tile([P, M], F32, tag="sq", name="squ", bufs=3)
            nc.scalar.activation(squ[:], u_b[:], AF.Square,
                                 bias=nuv[:, t, 0:1], scale=1.0)
            sqv = work.tile([P, M], F32, tag="sq", name="sqv", bufs=3)
            nc.scalar.activation(sqv[:], v_b[:], AF.Square,
                                 bias=nuv[:, t, 1:2], scale=1.0)
            # negate in place (exact), then negdm = (-squ) - sqv
            nc.scalar.activation(squ[:], squ[:], AF.Copy, bias=0.0, scale=-1.0)
            negdm = work.tile([P, M], F32, tag="negdm", name="negdm", bufs=3)
            nc.gpsimd.tensor_tensor(out=negdm[:], in0=squ[:], in1=sqv[:],
                                    op=OP.subtract)
            # top-16 threshold per row
            m1 = small.tile([P, 8], F32, tag="m1", name="m1")
            nc.vector.max(m1[:], negdm[:])
            mr = work.tile([P, M], F32, tag="mr", name="mr", bufs=2)
            nc.vector.match_replace(mr[:], m1[:], negdm[:], NEG_BIG)
            m2 = small.tile([P, 8], F32, tag="m2", name="m2")
            nc.vector.max(m2[:], mr[:])
            # queue this tile's mask (+-1 ACT Sign + x-bar transpose into wt)
            mask_q.append((negdm, m2, wt, tt))
            if len(mask_q) > 2:
                emit_mask()
        # matmuls for the previous group (its masks are all emitted by now)
        if prev_wt is not None:
            emit_matmuls(g - 1, prev_wt)
        prev_wt = wt

    while mask_q:
        emit_mask()
    emit_matmuls(T // GROUP - 1, prev_wt)
    for g in sorted(pend):
        fixup(g)

    # ---- eigen phase (slabs [P, T]) ----
    def et(name, shape=None):
        return epool.tile(shape or [P, T], F32, tag=name, name=name)

    vec = nc.vector

    def tt_(out, a, b, op):
        vec.tensor_tensor(out=out, in0=a, in1=b, op=op)

    S = et("S", [P, T, 9])
    tt_(S[:], cov[:, :, 0:9], cov[:, :, 9:18], OP.add)
    Sq = et("Sq", [P, T, 3])
    vec.tensor_scalar(out=Sq[:], in0=S[:, :, 0:3], scalar1=0.25, scalar2=None,
                      op0=OP.mult)
    cm = et("cm", [P, T, 6])
    tmp = et("tmp")
    for i, (a, b) in enumerate(pairs):
        tt_(tmp[:], Sq[:, :, a], Sq[:, :, b], OP.mult)
        tt_(cm[:, :, i], S[:, :, 3 + i], tmp[:], OP.subtract)

    cxx, cyy, czz = cm[:, :, 0], cm[:, :, 1], cm[:, :, 2]
    cxy, cxz, cyz = cm[:, :, 3], cm[:, :, 4], cm[:, :, 5]

    q = et("q")
    tt_(q[:], cxx, cyy, OP.add)
    tt_(q[:], q[:], czz, OP.add)
    vec.tensor_scalar(out=q[:], in0=q[:], scalar1=1.0 / 3.0, scalar2=None,
                      op0=OP.mult)
    b00, b11, b22 = et("b00"), et("b11"), et("b22")
    tt_(b00[:], cxx, q[:], OP.subtract)
    tt_(b11[:], cyy, q[:], OP.subtract)
    tt_(b22[:], czz, q[:], OP.subtract)
    p2 = et("p2")
    ta, tb = et("ta"), et("tb")
    tt_(p2[:], b00[:], b00[:], OP.mult)
    tt_(ta[:], b11[:], b11[:], OP.mult)
    tt_(p2[:], p2[:], ta[:], OP.add)
    tt_(ta[:], b22[:], b22[:], OP.mult)
    tt_(p2[:], p2[:], ta[:], OP.add)
    tt_(ta[:], cxy, cxy, OP.mult)
    tt_(tb[:], cxz, cxz, OP.mult)
    tt_(ta[:], ta[:], tb[:], OP.add)
    tt_(tb[:], cyz, cyz, OP.mult)
    tt_(ta[:], ta[:], tb[:], OP.add)
    vec.tensor_scalar(out=ta[:], in0=ta[:], scalar1=2.0, scalar2=None,
                      op0=OP.mult)
    tt_(p2[:], p2[:], ta[:], OP.add)
    p = et("p")
    nc.scalar.activation(p[:], p2[:], AF.Sqrt, bias=0.0, scale=1.0 / 6.0)
    pc = et("pc")
    vec.tensor_scalar(out=pc[:], in0=p[:], scalar1=1e-30, scalar2=None,
                      op0=OP.max)
    ip = et("ip")
    vec.reciprocal(ip[:], pc[:])
    p2x = et("p2x")
    vec.tensor_scalar(out=p2x[:], in0=p[:], scalar1=2.0, scalar2=None,
                      op0=OP.mult)
    # det(A - qI)
    det = et("det")
    tt_(ta[:], b11[:], b22[:], OP.mult)
    tt_(tb[:], cyz, cyz, OP.mult)
    tt_(ta[:], ta[:], tb[:], OP.subtract)
    tt_(det[:], b00[:], ta[:], OP.mult)
    tt_(ta[:], cxy, b22[:], OP.mult)
    tt_(tb[:], cyz, cxz, OP.mult)
    tt_(ta[:], ta[:], tb[:], OP.subtract)
    tt_(ta[:], cxy, ta[:], OP.mult)
    tt_(det[:], det[:], ta[:], OP.subtract)
    tt_(ta[:], cxy, cyz, OP.mult)
    tt_(tb[:], b11[:], cxz, OP.mult)
    tt_(ta[:], ta[:], tb[:], OP.subtract)
    tt_(ta[:], cxz, ta[:], OP.mult)
    tt_(det[:], det[:], ta[:], OP.add)
    # r = clamp(det * ip^3 / 2, -1, 1)
    r = et("r")
    tt_(ta[:], ip[:], ip[:], OP.mult)
    tt_(ta[:], ta[:], ip[:], OP.mult)
    tt_(r[:], det[:], ta[:], OP.mult)
    vec.tensor_scalar(out=r[:], in0=r[:], scalar1=0.5, scalar2=1.0,
                      op0=OP.mult, op1=OP.min)
    vec.tensor_scalar(out=r[:], in0=r[:], scalar1=-1.0, scalar2=None,
                      op0=OP.max)
    # acos(r) via octant-reduced arctan
    rr = et("rr")
    tt_(rr[:], r[:], r[:], OP.mult)
    aab = et("aab")
    nc.scalar.activation(aab[:], rr[:], AF.Sqrt, bias=0.0, scale=1.0)  # |r|
    vec.tensor_scalar(out=rr[:], in0=rr[:], scalar1=-1.0, scalar2=1.0,
                      op0=OP.mult, op1=OP.add)
    s = et("s")
    nc.scalar.activation(s[:], rr[:], AF.Sqrt, bias=0.0, scale=1.0)
    mn, mx = et("mn"), et("mx")
    tt_(mn[:], aab[:], s[:], OP.min)
    tt_(mx[:], aab[:], s[:], OP.max)
    imx = et("imx")
    vec.reciprocal(imx[:], mx[:])
    ratio = et("ratio")
    tt_(ratio[:], mn[:], imx[:], OP.mult)
    th = et("th")
    nc.scalar.activation(th[:], ratio[:], AF.Arctan, bias=0.0, scale=1.0)
    mk = et("mk")
    tt_(mk[:], s[:], aab[:], OP.is_gt)
    u1 = et("u1")
    vec.tensor_scalar(out=u1[:], in0=th[:], scalar1=-2.0, scalar2=PI / 2,
                      op0=OP.mult, op1=OP.add)
    tt_(u1[:], mk[:], u1[:], OP.mult)
    tt_(th[:], th[:], u1[:], OP.add)
    vec.tensor_scalar(out=mk[:], in0=r[:], scalar1=0.0, scalar2=None,
                      op0=OP.is_lt)
    vec.tensor_scalar(out=u1[:], in0=th[:], scalar1=-2.0, scalar2=PI,
                      op0=OP.mult, op1=OP.add)
    tt_(u1[:], mk[:], u1[:], OP.mult)
    tt_(th[:], th[:], u1[:], OP.add)
    phi = et("phi")
    vec.tensor_scalar(out=phi[:], in0=th[:], scalar1=1.0 / 3.0, scalar2=None,
                      op0=OP.mult)
    # cos(phi) = sin(phi + pi/2);  cos(phi + 2pi/3) = -sin(phi + pi/6)
    bias_c = et("bias_c", [P, 2])
    nc.gpsimd.memset(bias_c[:, 0:1], PI / 2)
    nc.gpsimd.memset(bias_c[:, 1:2], PI / 6)
    c1, c3 = et("c1"), et("c3")
    nc.scalar.activation(c1[:], phi[:], AF.Sin, bias=bias_c[:, 0:1], scale=1.0)
    nc.scalar.activation(c3[:], phi[:], AF.Sin, bias=bias_c[:, 1:2],
                         scale=1.0)
    eigs = et("eigs", [P, T, 3])
    tt_(ta[:], p2x[:], c1[:], OP.mult)
    tt_(eigs[:, :, 0], ta[:], q[:], OP.add)       # e1 = q + 2p*cos(phi)
    tt_(tb[:], p2x[:], c3[:], OP.mult)
    tt_(eigs[:, :, 2], q[:], tb[:], OP.subtract)  # e3 = q - 2p*sin(phi+pi/6)
    q3 = et("q3")
    vec.tensor_scalar(out=q3[:], in0=q[:], scalar1=3.0, scalar2=None,
                      op0=OP.mult)
    tt_(q3[:], q3[:], eigs[:, :, 0], OP.subtract)
    tt_(eigs[:, :, 1], q3[:], eigs[:, :, 2], OP.subtract)  # e2 = 3q - e1 - e3

    out_r = out_ap.rearrange("(t p) k -> p t k", p=P)
    for d in range(NDMA):
        sl = slice(d * tchunk, (d + 1) * tchunk)
        nc.sync.dma_start(out_r[:, sl, :], eigs[:, sl, :])


def _emit_with_ident(ctx, tc, out_ap, x_ap, uv_ap, M):
    # identity matrix (bf16) for the PE cov-transpose, built once
    nc = tc.nc
    const = ctx.enter_context(tc.tile_pool(name="identc", bufs=1))
    iota_a = const.tile([P, P], I32, tag="iota_a", name="iota_a")
    nc.gpsimd.iota(iota_a[:], pattern=[[1, P]], base=0, channel_multiplier=0)
    iota_b = const.tile([P, 1], I32, tag="iota_b", name="iota_b")
    nc.gpsimd.iota(iota_b[:], pattern=[[1, 1]], base=0, channel_multiplier=1)
    iota_af = const.tile([P, P], F32, tag="iota_af", name="iota_af")
    nc.gpsimd.tensor_copy(iota_af[:], iota_a[:])
    iota_bf = const.tile([P, 1], F32, tag="iota_bf", name="iota_bf")
    nc.gpsimd.tensor_copy(iota_bf[:], iota_b[:])
    ident = const.tile([P, P], F32, tag="ident", name="ident")
    nc.gpsimd.tensor_scalar(out=ident[:], in0=iota_af[:],
                            scalar1=iota_bf[:, 0:1],
                            scalar2=None, op0=OP.is_equal)
    _emit(ctx, tc, out_ap, x_ap, uv_ap, M, ident)


def build_nc(M: int = 4096):
    nc = bacc.Bacc("TRN2", target_bir_lowering=False, debug=False,
                   enable_asserts=False)
    x_ap = nc.dram_tensor("X", (M, 3), F32, kind="ExternalInput").ap()
    uv_ap = nc.dram_tensor("uv", (M, 2), F32, kind="ExternalInput").ap()
    out_ap = nc.dram_tensor("out", (M, 3), F32, kind="ExternalOutput").ap()
    with tile.TileContext(nc) as tc:
        with ExitStack() as ctx:
            _emit_with_ident(ctx, tc, out_ap, x_ap, uv_ap, M)
    nc.compile()
    return nc


_NC_CACHE = {}


def _get_nc(M: int = 4096):
    if M not in _NC_CACHE:
        _NC_CACHE[M] = build_nc(M)
    return _NC_CACHE[M]


def run(X, uv, trace: bool = False):
    B, M, _ = X.shape
    nc = _get_nc(M)
    in_maps = [
        {"X": np.ascontiguousarray(X[b], dtype=np.float32),
         "uv": np.ascontiguousarray(uv[b], dtype=np.float32)}
        for b in range(B)
    ]
    res = run_bass_kernel_spmd(nc, in_maps, core_ids=list(range(B)),
                               trace=trace)
    out = np.stack([r["out"] for r in res.results], axis=0)
    return out, res


def kernel(X, uv):
    X = np.asarray(X)
    uv = np.asarray(uv)
    out, _ = run(X, uv, trace=False)
    return out.astype(np.float32)


# revision 40
# speedup vs baseline: 1.0792x; 1.0046x over previous
"""Trainium2 Bass kernel for DiffGeomPropsApprox (within-batch uv-space 16-NN
-> neighborhood covariance of X -> descending symmetric-3x3 eigenvalues).

Sharding: data-parallel over batch B=8, one batch per NeuronCore (8 cores).

Per-core algorithm (M=4096 points):
  * negdm[i,j] = -((u_i-u_j)^2 + (v_i-v_j)^2), computed with the exact same
    f32 roundings as the reference (sub -> square -> add, negation is exact),
    so the NN ranking is bit-identical to the reference's top_k.
  * per query row: DVE max8 -> match_replace -> max8 gives t16 = the 16th
    largest negdm value; the selection mask is sign(negdm - t16_prev) in
    {-1,+1} bf16, produced on ACT (Sign LUT) so the DVE keeps only the sort
    unit work; sum_sel f = (sum +-f + sum_all f) / 2 fixes the polarity.
  * mask transposed via DMA x-bar transpose (idle DMA engines), cov sums via
    PE: feature-stationary matmuls (18 features = 9 cols x bf16 hi/lo; +-1
    mask is exact in bf16) accumulated in PSUM as [18, q].
  * cov = S2 - outer(S1)/16; closed-form symmetric 3x3 eigenvalues
    (trigonometric method; acos/cos built from Arctan/Sin/Sqrt LUTs).

Pipeline notes (these bought 3.3ms -> 0.69ms):
  * mask emission deferred 2 tiles, cov fixup deferred 2 groups, matmuls
    deferred 1 group: PSUM readers/late deps never head-of-line block the
    per-engine instruction streams.
  * GPSIMD tensor_scalar is pathologically slow (60us/pass) - never use it
    for big tiles; DVE tensor_scalar 2x mode stalls on the GPSIMD-shared
    SBUF port, so the mask lives on ACT instead.
  * partition-broadcast via log2 doubling SBUF->SBUF DMAs (step-0 DMA
    broadcast reads are ~10x slower).
"""

from contextlib import ExitStack

import numpy as np

import concourse.bass as bass
import concourse.tile as tile
from concourse import bacc, mybir
from concourse.alu_op_type import AluOpType
from concourse.bass_utils import run_bass_kernel_spmd

F32 = mybir.dt.float32
BF16 = mybir.dt.bfloat16
I32 = mybir.dt.int32
AF = mybir.ActivationFunctionType
OP = AluOpType

P = 128
K = 16
GROUP = 2           # query tiles per matmul group
NEG_BIG = -3.0e38
PI = float(np.pi)


def _emit(ctx: ExitStack, tc, out_ap, x_ap, uv_ap, M: int, ident18):
    nc = tc.nc
    T = M // P          # number of 128-row tiles
    NF = 18             # 9 features x (hi, lo)
    NDMA = min(8, T)    # split slow strided DMAs this many ways

    const = ctx.enter_context(tc.tile_pool(name="const", bufs=1))
    work = ctx.enter_context(tc.tile_pool(name="work", bufs=2))
    small = ctx.enter_context(tc.tile_pool(name="small", bufs=4))
    psum = ctx.enter_context(tc.tile_pool(name="psum", bufs=2, space="PSUM"))
    epool = ctx.enter_context(tc.tile_pool(name="eig", bufs=1))

    # ---- broadcast candidate coords across partitions (doubling DMAs) ----
    u_b = const.tile([P, M], F32, tag="u_b")
    v_b = const.tile([P, M], F32, tag="v_b")
    uv_t = uv_ap.rearrange("m k -> k m")
    nc.sync.dma_start(u_b[0:1, :], uv_t[0:1, :])
    nc.sync.dma_start(v_b[0:1, :], uv_t[1:2, :])
    k = 1
    while k < P:
        nc.sync.dma_start(u_b[k:2 * k, :], u_b[0:k, :])
        nc.sync.dma_start(v_b[k:2 * k, :], v_b[0:k, :])
        k *= 2

    # slab layouts [P, T, k]: point m = t*128 + p  (strided loads, split)
    uv_slab = const.tile([P, T, 2], F32, tag="uv_slab")
    uv_r = uv_ap.rearrange("(t p) k -> p t k", p=P)
    x_slab = const.tile([P, T, 3], F32, tag="x_slab")
    x_r = x_ap.rearrange("(t p) k -> p t k", p=P)
    tchunk = T // NDMA
    for d in range(NDMA):
        sl = slice(d * tchunk, (d + 1) * tchunk)
        nc.sync.dma_start(uv_slab[:, sl, :], uv_r[:, sl, :])
        nc.sync.dma_start(x_slab[:, sl, :], x_r[:, sl, :])
    nuv = const.tile([P, T, 2], F32, tag="nuv")
    nc.vector.tensor_scalar(out=nuv[:], in0=uv_slab[:], scalar1=-1.0,
                            scalar2=None, op0=OP.mult)

    # ---- features: [x y z x2 y2 z2 xy xz yz] as bf16 hi/lo ----
    pairs = [(0, 0), (1, 1), (2, 2), (0, 1), (0, 2), (1, 2)]
    fsl = work.tile([P, T, 9], F32, tag="fsl", name="fsl", bufs=1)
    nc.vector.tensor_copy(fsl[:, :, 0:3], x_slab[:])
    for i, (a, b) in enumerate(pairs):
        nc.vector.tensor_tensor(out=fsl[:, :, 3 + i], in0=x_slab[:, :, a],
                                in1=x_slab[:, :, b], op=OP.mult)
    fbf = const.tile([P, T, NF], BF16, tag="fbf")
    nc.vector.tensor_copy(fbf[:, :, 0:9], fsl[:])
    fhi32 = work.tile([P, T, 9], F32, tag="fhi32", name="fhi32", bufs=1)
    nc.vector.tensor_copy(fhi32[:], fbf[:, :, 0:9])
    nc.vector.tensor_tensor(out=fbf[:, :, 9:18], in0=fsl[:], in1=fhi32[:],
                            op=OP.subtract)

    cov = const.tile([P, T, NF], F32, tag="cov")

    # per-feature totals sum_c F[c, f] (for the +-1 sign-mask trick):
    # acc = sum_sel f - sum_unsel f  =>  sum_sel = (acc + ftot) / 2
    ones_c = const.tile([P, 1], BF16, tag="ones_c")
    nc.gpsimd.memset(ones_c[:], 1.0)
    ftot_ps = psum.tile([NF, 1], F32, tag="ftot", name="ftot_ps", bufs=1)
    for j in range(T):
        nc.tensor.matmul(ftot_ps[:], lhsT=fbf[:, j, :], rhs=ones_c[:],
                         start=(j == 0), stop=(j == T - 1))
    ftot_h = const.tile([NF, 1], F32, tag="ftot_h")
    nc.vector.tensor_scalar(out=ftot_h[:], in0=ftot_ps[:], scalar1=0.5,
                            scalar2=None, op0=OP.mult)

    # ---- main loop over query tiles, grouped for the matmul phase ----
    # The cov fixup (PSUM acc -> SBUF -> PE transpose -> cov slab) for group g
    # is emitted two groups later so its PSUM reads never head-of-line block
    # the ACT/PE streams of the current group.
    assert T % GROUP == 0
    pend = {}

    def fixup(g):
        acc_g = pend.pop(g)
        covg = work.tile([NF, P * GROUP], F32, tag="covg", name="covg", bufs=1)
        nc.vector.tensor_scalar(out=covg[:], in0=acc_g[:], scalar1=0.5,
                                scalar2=ftot_h[:], op0=OP.mult, op1=OP.add)
        for tt in range(GROUP):
            t = g * GROUP + tt
            ctp = psum.tile([P, NF], F32, tag="ctp", name="ctp", bufs=3)
            nc.tensor.matmul(ctp[:], lhsT=covg[:, tt * P:(tt + 1) * P],
                             rhs=ident18[0:NF, 0:NF], is_transpose=True)
            nc.vector.tensor_copy(cov[:, t, :], ctp[:])

    # mask emission for tile t is deferred into tile t+1 so the ACT Sign pass
    # never head-of-line blocks the next tile's Square passes.
    mask_q = []

    def emit_mask():
        negdm_p, m2_p, wt_p, tt_p = mask_q.pop(0)
        nt16p = small.tile([P, 1], F32, tag="nt16p", name="nt16p")
        nc.vector.tensor_scalar(out=nt16p[:], in0=m2_p[:, 7:8],
                                scalar1=-(1.0 + 2.0 ** -22), scalar2=None,
                                op0=OP.mult)
        wmask = work.tile([P, M], BF16, tag="wmask", name="wmask")
        nc.scalar.activation(wmask[:], negdm_p[:], AF.Sign,
                             bias=nt16p[:], scale=1.0)
        nc.sync.dma_start(wt_p[:, :, tt_p * P:(tt_p + 1) * P], wmask[:],
                          transpose=True)

    def emit_matmuls(g, wt_g):
        acc = psum.tile([NF, P * GROUP], F32, tag="acc", name="acc", bufs=4)
        for j in range(T):
            nc.tensor.matmul(acc[:], lhsT=fbf[:, j, :], rhs=wt_g[:, j, :],
                             start=(j == 0), stop=(j == T - 1))
        pend[g] = acc

    prev_wt = None
    for g in range(T // GROUP):
        if g >= 2:
            fixup(g - 2)
        wt = work.tile([P, T, P * GROUP], BF16, tag="wt", name="wt", bufs=1)
        for tt in range(GROUP):
            t = g * GROUP + tt
            # exact f32 squared diffs: ACT Square with per-partition bias -u_q
            squ = work.tile([P, M], F32, tag="sq", name="squ", bufs=3)
            nc.scalar.activation(squ[:], u_b[:], AF.Square,
                                 bias=nuv[:, t, 0:1], scale=1.0)
            sqv = work.tile([P, M], F32, tag="sq", name="sqv", bufs=3)
            nc.scalar.activation(sqv[:], v_b[:], AF.Square,
                                 bias=nuv[:, t, 1:2], scale=1.0)
            # negate in place (exact), then negdm = (-squ) - sqv
            nc.scalar.activation(squ[:], squ[:], AF.Copy, bias=0.0, scale=-1.0)
            negdm = work.tile([P, M], F32, tag="negdm", name="negdm", bufs=3)
            nc.gpsimd.tensor_tensor(out=negdm[:], in0=squ[:], in1=sqv[:],
                                    op=OP.subtract)
            # top-16 threshold per row
            m1 = small.tile([P, 8], F32, tag="m1", name="m1")
            nc.vector.max(m1[:], negdm[:])
            mr = work.tile([P, M], F32, tag="mr", name="mr", bufs=2)
            nc.vector.match_replace(mr[:], m1[:], negdm[:], NEG_BIG)
            m2 = small.tile([P, 8], F32, tag="m2", name="m2")
            nc.vector.max(m2[:], mr[:])
            # queue this tile's mask (+-1 ACT Sign + x-bar transpose into wt)
            mask_q.append((negdm, m2, wt, tt))
            if len(mask_q) > 2:
                emit_mask()
        # matmuls for the previous group (its masks are all emitted by now)
        if prev_wt is not None:
            emit_matmuls(g - 1, prev_wt)
        prev_wt = wt

    while mask_q:
        emit_mask()
    emit_matmuls(T // GROUP - 1, prev_wt)
    for g in sorted(pend):
        fixup(g)

    # ---- eigen phase (slabs [P, T]) ----
    def et(name, shape=None):
        return epool.tile(shape or [P, T], F32, tag=name, name=name)

    vec = nc.vector

    def tt_(out, a, b, op):
        vec.tensor_tensor(out=out, in0=a, in1=b, op=op)

    S = et("S", [P, T, 9])
    tt_(S[:], cov[:, :, 0:9], cov[:, :, 9:18], OP.add)
    Sq = et("Sq", [P, T, 3])
    vec.tensor_scalar(out=Sq[:], in0=S[:, :, 0:3], scalar1=0.25, scalar2=None,
                      op0=OP.mult)
    cm = et("cm", [P, T, 6])
    tmp = et("tmp")
    for i, (a, b) in enumerate(pairs):
        tt_(tmp[:], Sq[:, :, a], Sq[:, :, b], OP.mult)
        tt_(cm[:, :, i], S[:, :, 3 + i], tmp[:], OP.subtract)

    cxx, cyy, czz = cm[:, :, 0], cm[:, :, 1], cm[:, :, 2]
    cxy, cxz, cyz = cm[:, :, 3], cm[:, :, 4], cm[:, :, 5]

    q = et("q")
    tt_(q[:], cxx, cyy, OP.add)
    tt_(q[:], q[:], czz, OP.add)
    vec.tensor_scalar(out=q[:], in0=q[:], scalar1=1.0 / 3.0, scalar2=None,
                      op0=OP.mult)
    b00, b11, b22 = et("b00"), et("b11"), et("b22")
    tt_(b00[:], cxx, q[:], OP.subtract)
    tt_(b11[:], cyy, q[:], OP.subtract)
    tt_(b22[:], czz, q[:], OP.subtract)
    p2 = et("p2")
    ta, tb = et("ta"), et("tb")
    tt_(p2[:], b00[:], b00[:], OP.mult)
    tt_(ta[:], b11[:], b11[:], OP.mult)
    tt_(p2[:], p2[:], ta[:], OP.add)
    tt_(ta[:], b22[:], b22[:], OP.mult)
    tt_(p2[:], p2[:], ta[:], OP.add)
    tt_(ta[:], cxy, cxy, OP.mult)
    tt_(tb[:], cxz, cxz, OP.mult)
    tt_(ta[:], ta[:], tb[:], OP.add)
    tt_(tb[:], cyz, cyz, OP.mult)
    tt_(ta[:], ta[:], tb[:], OP.add)
    vec.tensor_scalar(out=ta[:], in0=ta[:], scalar1=2.0, scalar2=None,
                      op0=OP.mult)
    tt_(p2[:], p2[:], ta[:], OP.add)
    p = et("p")
    nc.scalar.activation(p[:], p2[:], AF.Sqrt, bias=0.0, scale=1.0 / 6.0)
    pc = et("pc")
    vec.tensor_scalar(out=pc[:], in0=p[:], scalar1=1e-30, scalar2=None,
                      op0=OP.max)
    ip = et("ip")
    vec.reciprocal(ip[:], pc[:])
    p2x = et("p2x")
    vec.tensor_scalar(out=p2x[:], in0=p[:], scalar1=2.0, scalar2=None,
                      op0=OP.mult)
    # det(A - qI)
    det = et("det")
    tt_(ta[:], b11[:], b22[:], OP.mult)
    tt_(tb[:], cyz, cyz, OP.mult)
    tt_(ta[:], ta[:], tb[:], OP.subtract)
    tt_(det[:], b00[:], ta[:], OP.mult)
    tt_(ta[:], cxy, b22[:], OP.mult)
    tt_(tb[:], cyz, cxz, OP.mult)
    tt_(ta[:], ta[:], tb[:], OP.subtract)
    tt_(ta[:], cxy, ta[:], OP.mult)
    tt_(det[:], det[:], ta[:], OP.subtract)
    tt_(ta[:], cxy, cyz, OP.mult)
    tt_(tb[:], b11[:], cxz, OP.mult)
    tt_(ta[:], ta[:], tb[:], OP.subtract)
    tt_(ta[:], cxz, ta[:], OP.mult)
    tt_(det[:], det[:], ta[:], OP.add)
    # r = clamp(det * ip^3 / 2, -1, 1)
    r = et("r")
    tt_(ta[:], ip[:], ip[:], OP.mult)
    tt_(ta[:], ta[:], ip[:], OP.mult)
    tt_(r[:], det[:], ta[:], OP.mult)
    vec.tensor_scalar(out=r[:], in0=r[:], scalar1=0.5, scalar2=1.0,
                      op0=OP.mult, op1=OP.min)
    vec.tensor_scalar(out=r[:], in0=r[:], scalar1=-1.0, scalar2=None,
                      op0=OP.max)
    # acos(r) via octant-reduced arctan
    rr = et("rr")
    tt_(rr[:], r[:], r[:], OP.mult)
    aab = et("aab")
    nc.scalar.activation(aab[:], rr[:], AF.Sqrt, bias=0.0, scale=1.0)  # |r|
    vec.tensor_scalar(out=rr[:], in0=rr[:], scalar1=-1.0, scalar2=1.0,
                      op0=OP.mult, op1=OP.add)
    s = et("s")
    nc.scalar.activation(s[:], rr[:], AF.Sqrt, bias=0.0, scale=1.0)
    mn, mx = et("mn"), et("mx")
    tt_(mn[:], aab[:], s[:], OP.min)
    tt_(mx[:], aab[:], s[:], OP.max)
    imx = et("imx")
    vec.reciprocal(imx[:], mx[:])
    ratio = et("ratio")
    tt_(ratio[:], mn[:], imx[:], OP.mult)
    th = et("th")
    nc.scalar.activation(th[:], ratio[:], AF.Arctan, bias=0.0, scale=1.0)
    mk = et("mk")
    tt_(mk[:], s[:], aab[:], OP.is_gt)
    u1 = et("u1")
    vec.tensor_scalar(out=u1[:], in0=th[:], scalar1=-2.0, scalar2=PI / 2,
                      op0=OP.mult, op1=OP.add)
    tt_(u1[:], mk[:], u1[:], OP.mult)
    tt_(th[:], th[:], u1[:], OP.add)
    vec.tensor_scalar(out=mk[:], in0=r[:], scalar1=0.0, scalar2=None,
                      op0=OP.is_lt)
    vec.tensor_scalar(out=u1[:], in0=th[:], scalar1=-2.0, scalar2=PI,
                      op0=OP.mult, op1=OP.add)
    tt_(u1[:], mk[:], u1[:], OP.mult)
    tt_(th[:], th[:], u1[:], OP.add)
    phi = et("phi")
    vec.tensor_scalar(out=phi[:], in0=th[:], scalar1=1.0 / 3.0, scalar2=None,
                      op0=OP.mult)
    # cos(phi) = sin(phi + pi/2);  cos(phi + 2pi/3) = -sin(phi + pi/6)
    bias_c = et("bias_c", [P, 2])
    nc.gpsimd.memset(bias_c[:, 0:1], PI / 2)
    nc.gpsimd.memset(bias_c[:, 1:2], PI / 6)
    c1, c3 = et("c1"), et("c3")
    nc.scalar.activation(c1[:], phi[:], AF.Sin, bias=bias_c[:, 0:1], scale=1.0)
    nc.scalar.activation(c3[:], phi[:], AF.Sin, bias=bias_c[:, 1:2],
                         scale=1.0)
    eigs = et("eigs", [P, T, 3])
    tt_(ta[:], p2x[:], c1[:], OP.mult)
    tt_(eigs[:, :, 0], ta[:], q[:], OP.add)       # e1 = q + 2p*cos(phi)
    tt_(tb[:], p2x[:], c3[:], OP.mult)
    tt_(eigs[:, :, 2], q[:], tb[:], OP.subtract)  # e3 = q - 2p*sin(phi+pi/6)
    q3 = et("q3")
    vec.tensor_scalar(out=q3[:], in0=q[:], scalar1=3.0, scalar2=None,
                      op0=OP.mult)
    tt_(q3[:], q3[:], eigs[:, :, 0], OP.subtract)
    tt_(eigs[:, :, 1], q3[:], eigs[:, :, 2], OP.subtract)  # e2 = 3q - e1 - e3

    out_r = out_ap.rearrange("(t p) k -> p t k", p=P)
    for d in range(NDMA):
        sl = slice(d * tchunk, (d + 1) * tchunk)
        nc.sync.dma_start(out_r[:, sl, :], eigs[:, sl, :])


def _emit_with_ident(ctx, tc, out_ap, x_ap, uv_ap, M):
    # identity matrix (bf16) for the PE cov-transpose, built once
    nc = tc.nc
    const = ctx.enter_context(tc.tile_pool(name="identc", bufs=1))
    iota_a = const.tile([P, P], I32, tag="iota_a", name="iota_a")
    nc.gpsimd.iota(iota_a[:], pattern=[[1, P]], base=0, channel_multiplier=0)
    iota_b = const.tile([P, 1], I32, tag="iota_b", name="iota_b")
    nc.gpsimd.iota(iota_b[:], pattern=[[1, 1]], base=0, channel_multiplier=1)
    iota_af = const.tile([P, P], F32, tag="iota_af", name="iota_af")
    nc.gpsimd.tensor_copy(iota_af[:], iota_a[:])
    iota_bf = const.tile([P, 1], F32, tag="iota_bf", name="iota_bf")
    nc.gpsimd.tensor_copy(iota_bf[:], iota_b[:])
    ident = const.tile([P, P], F32, tag="ident", name="ident")
    nc.gpsimd.tensor_scalar(out=ident[:], in0=iota_af[:],
                            scalar1=iota_bf[:, 0:1],
                            scalar2=None, op0=OP.is_equal)
    _emit(ctx, tc, out_ap, x_ap, uv_ap, M, ident)


def build_nc(M: int = 4096):
    nc = bacc.Bacc("TRN2", target_bir_lowering=False, debug=False,
                   enable_asserts=False)
    x_ap = nc.dram_tensor("X", (M, 3), F32, kind="ExternalInput").ap()
    uv_ap = nc.dram_tensor("uv", (M, 2), F32, kind="ExternalInput").ap()
    out_ap = nc.dram_tensor("out", (M, 3), F32, kind="ExternalOutput").ap()
    with tile.TileContext(nc) as tc:
        with ExitStack() as ctx:
            _emit_with_ident(ctx, tc, out_ap, x_ap, uv_ap, M)
    nc.compile()
    return nc


_NC_CACHE = {}


def _get_nc(M: int = 4096):
    if M not in _NC_CACHE:
        _NC_CACHE[M] = build_nc(M)
    return _NC_CACHE[M]


def run(X, uv, trace: bool = False):
    B, M, _ = X.shape
    nc = _get_nc(M)
    in_maps = [
        {"X": np.ascontiguousarray(X[b], dtype=np.float32),
         "uv": np.ascontiguousarray(uv[b], dtype=np.float32)}
        for b in range(B)
    ]
    res = run_bass_kernel_spmd(nc, in_maps, core_ids=list(range(B)),
                               trace=trace)
    out = np.stack([r["out"] for r in res.results], axis=0)
    return out, res


def kernel(X, uv):
    X = np.asarray(X)
    uv = np.asarray(uv)
    out, _ = run(X, uv, trace=False)
    return out.astype(np.float32)
